# revision 24
# baseline (speedup 1.0000x reference)
"""
Trainium2 Bass kernel for a 2-layer GCN (nn_Method_GCN_81243601371888).

  out = log_softmax( A @ relu(A @ (x@W1) + b1) @ W2 + b2 )

where A = D^-1/2 (Adj + I) D^-1/2 is the symmetric-normalized adjacency
(with self loops), N=100000 nodes, E=3.2M directed edges.

Strategy (8 NeuronCores, full inputs in / full output out):

  * Nodes are sharded contiguously: NC n owns dst nodes [n*12500, (n+1)*12500).
  * norm[e] = dinv[src]*dinv[dst] is separable, so the per-edge aggregation
    becomes: zt = dinv * h;  agg[d] = sum_{e->d} zt[src];  out = dinv*agg.
    No per-edge scaling needed on-chip.
  * h = x@W1 is computed shard-locally (TensorE), scaled by dinv, then
    AllGathered so every NC holds zt for all nodes: SBUF tile [128, NPAD]
    where partition 16*q+f holds feature f of NC-q's nodes ("octant" layout).
  * The per-edge gather runs on GPSIMD ap_gather: Q7 core q of each NC
    handles edges whose src lives in octant q (its own 16 partitions),
    with per-core index streams.  Messages are emitted directly in
    dst-grouped padded-CSR order (grouped by exact per-(NC,octant) degree
    class), so the segment-sum is a strided DVE tensor_reduce - no scatter.
  * Per-octant partial sums are then aligned to the canonical node order
    with a second (small) ap_gather and folded across the 8 octant
    partition groups with a single [128->16] matmul on TensorE.
  * Layer 2 aggregates the 16-wide relu output (A@r) and applies W2 after,
    reusing the identical index streams; then bias + log_softmax on-chip.

Host-side work is limited to index preprocessing (degree counts, edge
bucketing/sorting, int16 index stream construction) and input layout
(x transposed per shard); all FLOPs and all per-edge data movement run
on the NeuronCores.

Performance notes (measured on TRN2):
  * The wall is the GPSIMD ap_gather: ~29 ns per index (2 indices per
    Q7 read command at ~102 cycles command latency, independent of d or
    call size), so phase-1 gathers ~3.2 ms + align gathers ~0.5 ms
    dominate; everything else overlaps underneath.
  * x/W1 are fed to the x@W1 matmul in bf16 (halves the 25 MB/core HBM
    read; ~2e-4 relative error overall).
  * log_softmax is batched: all per-block Exp (one table load), one Ln,
    then per-block adds, avoiding the per-block activation-table reload
    thrash (~1.3 us per ACT_TABLE_LOAD).
"""

import os
import sys
import numpy as np
from contextlib import ExitStack
from dataclasses import dataclass, field

for _p in ("/opt/trn_rl_repo",):
    if _p not in sys.path and os.path.isdir(_p):
        sys.path.insert(0, _p)

import concourse.bass as bass
import concourse.bacc as bacc
import concourse.mybir as mybir
import concourse.tile as tile
from concourse._compat import with_exitstack

F32 = mybir.dt.float32
BF16 = mybir.dt.bfloat16
I16 = mybir.dt.int16


# --------------------------------------------------------------------------
# configuration
# --------------------------------------------------------------------------
@dataclass
class Cfg:
    N: int = 100000          # nodes
    FIN: int = 500           # input features
    H: int = 16              # hidden features
    CLS: int = 3             # classes
    NCORES: int = 8          # NeuronCores
    TILE: int = 4096         # msgs megatile slots (per Q7 core)
    MMCH: int = 512          # matmul free-dim chunk
    ALCH: int = 1024         # align-gather chunk (nodes)

    @property
    def NSH(self):           # nodes per NC shard
        return self.N // self.NCORES

    @property
    def NPAD(self):          # shard padded to a multiple of 128
        return ((self.NSH + 127) // 128) * 128

    DEBUG: bool = False      # add intermediate dump outputs
    REPEAT: int = 1          # run the whole body R times (for timing)
    FAKE_CC: bool = False    # timing only: replace AllGather with local DMA
    SKIP_PHASE0: bool = False  # debug: feed zloc from host, skip x@W1
    SKIP_PHASE2: bool = False  # debug: stop after phase-1 (layer 0 only)
    SKIP_REDUCE: bool = False  # debug: no memset/tensor_reduce in phase-1
    SKIP_GATHER: bool = False  # debug: no ap_gather in phase-1

    @property
    def KT(self):            # contraction tiles for x@W1
        return (self.FIN + 127) // 128

    @property
    def KP(self):            # contraction partition size
        return (self.FIN + self.KT - 1) // self.KT


# --------------------------------------------------------------------------
# host preprocessing: degree classes, index streams
# --------------------------------------------------------------------------
@dataclass
class Struct:
    S: int                    # per-core slot-stream length
    P_sz: int                 # partial buffer width (incl zero col 0)
    pos: np.ndarray           # pos[k] = partial col of first class-k node
    megatiles: list = field(default_factory=list)
    # megatile: (i0_cols, nidx, pieces) ; piece: (k, m0, mcnt, off_slots)
    idxp1: np.ndarray = None  # [NC][128, S//16] int16
    idxal: np.ndarray = None  # [NC][128, NPAD//16] int16
    dinv: np.ndarray = None   # [N] f32


def _ceil16(v):
    return ((int(v) + 15) // 16) * 16


def _ceil32(v):
    # class sizes are kept to multiples of 32 nodes so every piece (and
    # therefore every ap_gather stream) is a multiple of 32 slots: the Q7
    # ucode reads the int16 index stream as 4-byte words, so each
    # instruction's index base must stay 4-byte aligned.
    return ((int(v) + 31) // 32) * 32


def preprocess(edge_index: np.ndarray, cfg: Cfg) -> Struct:
    N, NC, NSH, NPAD = cfg.N, cfg.NCORES, cfg.NSH, cfg.NPAD
    src = np.asarray(edge_index[0], dtype=np.int64)
    dst = np.asarray(edge_index[1], dtype=np.int64)

    # self-loops are handled analytically on-device (agg += zt), not as edges
    deg = (np.bincount(dst, minlength=N) + 1).astype(np.float64)
    dinv = (1.0 / np.sqrt(np.maximum(deg, 1.0))).astype(np.float32)

    n_of = dst // NSH                       # owning NC of the dst
    c_of = src // NSH                       # src octant == Q7 core
    sloc = (src % NSH).astype(np.int64)
    dloc = (dst % NSH).astype(np.int64)
    grp = n_of * NC + c_of                  # 0..63
    key = grp * NSH + dloc                  # per-(NC,octant,dst-node) bin

    cnt = np.bincount(key, minlength=NC * NC * NSH)   # per-node degree in grp
    kmax = int(cnt.max())

    # quantized degree classes: exact up to 8, then even, then coarser
    Q = [q for q in range(1, kmax + 1)
         if q <= 8 or (q <= 16 and q % 2 == 0) or q % 4 == 0]
    if Q[-1] < kmax:
        Q.append(_ceil16(kmax) if kmax > 16 else kmax)
    Q = np.array(Q, dtype=np.int64)
    NQ = len(Q)
    k2qi = np.zeros(kmax + 1, dtype=np.int64)         # degree -> class index
    k2qi[1:] = np.searchsorted(Q, np.arange(1, kmax + 1))
    qi = k2qi[cnt]                                    # class idx of each node
    qsz = Q[qi]                                       # padded degree of node

    # global class sizes M[qi], padded to multiples of 16
    gq = (key // NSH) * NQ + 0                        # placeholder
    node_grp = np.repeat(np.arange(NC * NC), NSH)
    occ_mask = cnt > 0
    Mcnt = np.bincount(node_grp[occ_mask] * NQ + qi[occ_mask],
                       minlength=NC * NC * NQ).reshape(NC * NC, NQ)
    # class blocks must stay multiples of 32 slots (4-byte alignment of the
    # wrapped int16 index streams): even k only needs M%16, odd k needs M%32
    M = np.array([_ceil16(v) if Q[i] % 2 == 0 else _ceil32(v)
                  for i, v in enumerate(Mcnt.max(axis=0))], dtype=np.int64)
    pos = np.zeros(NQ + 1, dtype=np.int64)            # partial column bases
    pos[0] = 1
    for i in range(NQ):
        pos[i + 1] = pos[i] + M[i]
    P_sz = int(pos[NQ])
    base = np.zeros(NQ + 1, dtype=np.int64)           # slot-stream bases
    for i in range(NQ):
        base[i + 1] = base[i] + M[i] * Q[i]
    S = int(base[NQ])
    assert S % 16 == 0 and P_sz < 32768 and NPAD < 32768

    # per-node rank within its (grp, class), ordered by dloc
    occ = np.flatnonzero(occ_mask)                    # occupied (grp,node) keys
    og, od = occ // NSH, occ % NSH
    oq = qi[occ]
    order_n = np.lexsort((od, oq, og))                # by (grp, class, dloc)
    sg, sk = og[order_n], oq[order_n]
    new_gk = np.r_[True, (sg[1:] != sg[:-1]) | (sk[1:] != sk[:-1])]
    gk_start = np.flatnonzero(new_gk)
    rank_sorted = np.arange(len(occ)) - np.repeat(
        gk_start, np.diff(np.r_[gk_start, len(occ)])
    )
    rank = np.zeros(NC * NC * NSH, dtype=np.int64)
    rank[occ[order_n]] = rank_sorted

    # within-node edge index j (stable order of edges per key)
    order_e = np.argsort(key, kind="stable")
    ks = key[order_e]
    new_k = np.r_[True, ks[1:] != ks[:-1]]
    k_start = np.flatnonzero(new_k)
    j_sorted = np.arange(len(ks)) - np.repeat(
        k_start, np.diff(np.r_[k_start, len(ks)])
    )
    j = np.empty(len(ks), dtype=np.int64)
    j[order_e] = j_sorted

    kq = qi[key]                                      # class idx of each edge
    slot = base[kq] + rank[key] * Q[kq] + j           # slot within (grp) stream

    ZIDX = NSH                                        # a guaranteed-zero column
    idx_stream = np.full((NC * NC, S), ZIDX, dtype=np.int16)
    idx_stream[grp, slot] = sloc.astype(np.int16)

    # wrapped [16, S//16] per core, stacked to [128, S//16] per NC
    idxp1 = np.zeros((NC, 128, S // 16), dtype=np.int16)
    for n in range(NC):
        for c in range(NC):
            w = idx_stream[n * NC + c].reshape(S // 16, 16).T
            idxp1[n, 16 * c:16 * (c + 1)] = w

    # align indices: node j of shard -> partial column (or 0)
    al = np.zeros((NC * NC, NPAD), dtype=np.int16)
    pcol = pos[qi[occ]] + rank[occ]                   # partial col per occupied
    al[og, od] = pcol.astype(np.int16)
    idxal = np.zeros((NC, 128, NPAD // 16), dtype=np.int16)
    for n in range(NC):
        for c in range(NC):
            w = al[n * NC + c].reshape(NPAD // 16, 16).T
            idxal[n, 16 * c:16 * (c + 1)] = w

    # megatiles: split class blocks into pieces of <= TILE slots
    pieces = []
    for i in range(NQ):
        k = int(Q[i])
        m0 = 0
        step = max(32, (cfg.TILE // k) // 32 * 32)
        while m0 < M[i]:
            mcnt = min(step, M[i] - m0)
            pieces.append((i, k, m0, int(mcnt)))
            m0 += mcnt
    megatiles = []
    cur, cur_slots = [], 0
    slot_cursor = 0
    for (i, k, m0, mcnt) in pieces:
        sl = mcnt * k
        if cur and cur_slots + sl > cfg.TILE:
            megatiles.append((slot_cursor - cur_slots, cur_slots, cur))
            cur, cur_slots = [], 0
        cur.append((int(pos[i]) + m0, k, mcnt, cur_slots))
        cur_slots += sl
        slot_cursor += sl
    if cur:
        megatiles.append((slot_cursor - cur_slots, cur_slots, cur))

    st = Struct(S=S, P_sz=P_sz, pos=pos, megatiles=megatiles,
                idxp1=idxp1, idxal=idxal, dinv=dinv)
    return st


# --------------------------------------------------------------------------
# device program
# --------------------------------------------------------------------------
@with_exitstack
def gcn_body(ctx: ExitStack, tc: "tile.TileContext", T: dict, cfg: Cfg,
             st: Struct):
    nc = tc.nc
    H, CLS, NPAD, MMCH, ALCH = cfg.H, cfg.CLS, cfg.NPAD, cfg.MMCH, cfg.ALCH
    KT, KP = cfg.KT, cfg.KP
    NBLK = NPAD // 128                    # 128-node blocks for the final mm

    sb = ctx.enter_context(tc.tile_pool(name="sb", bufs=1))
    xp = ctx.enter_context(tc.tile_pool(name="xp", bufs=KT + 1))
    dp = ctx.enter_context(tc.tile_pool(name="dp", bufs=2))
    zc = ctx.enter_context(tc.tile_pool(name="zc", bufs=2))
    mg = ctx.enter_context(tc.tile_pool(name="mg", bufs=2))
    alp = ctx.enter_context(tc.tile_pool(name="al", bufs=2))
    sm = ctx.enter_context(tc.tile_pool(name="sm", bufs=4))
    psh = ctx.enter_context(tc.tile_pool(name="psh", bufs=2, space="PSUM"))
    psf = ctx.enter_context(tc.tile_pool(name="psf", bufs=2, space="PSUM"))
    ps3 = ctx.enter_context(tc.tile_pool(name="ps3", bufs=2, space="PSUM"))

    # persistent tiles
    idxp1 = sb.tile([128, st.S // 16], I16)
    nc.sync.dma_start(idxp1[:], T["idxp1"][:])
    zbuf = sb.tile([128, NPAD], F32)
    w1 = sb.tile([KP, KT * H], BF16)
    nc.sync.dma_start(w1[:], T["W1s"][:])
    w2 = sb.tile([H, CLS], F32)
    nc.sync.dma_start(w2[:], T["W2s"][:])
    b1 = sb.tile([H, 1], F32)
    nc.sync.dma_start(b1[:], T["b1s"][:])
    b2t = sb.tile([128, CLS], F32)
    nc.sync.dma_start(b2t[:], T["b2t"][:])
    foldm = sb.tile([128, H], F32)
    nc.sync.dma_start(foldm[:], T["foldm"][:])
    idxal = sb.tile([128, NPAD // 16], I16)
    nc.sync.dma_start(idxal[:], T["idxal"][:])

    partial = sb.tile([128, st.P_sz], F32)
    outsb = sb.tile([128, NBLK * CLS], F32)
    # batched log_softmax state (one activation function at a time, so the
    # Scalar engine loads each activation table once instead of per block)
    mxall = sb.tile([128, NBLK], F32)
    seall = sb.tile([128, NBLK], F32)
    lsall = sb.tile([128, NBLK], F32)
    uall = sb.tile([128, NBLK], F32)

    def mm_chunks():
        for c0 in range(0, NPAD, MMCH):
            yield c0, min(MMCH, NPAD - c0)

    for _rep in range(cfg.REPEAT):
        _gcn_once(nc, tc, T, cfg, st, locals())


def _gcn_once(nc, tc, T, cfg, st, env):
    H, CLS, NPAD, MMCH, ALCH = cfg.H, cfg.CLS, cfg.NPAD, cfg.MMCH, cfg.ALCH
    KT, KP = cfg.KT, cfg.KP
    NBLK = NPAD // 128
    sb, xp, dp, zc, mg, alp, sm = (env[k] for k in
                                   ("sb", "xp", "dp", "zc", "mg", "alp", "sm"))
    psh, psf, ps3 = env["psh"], env["psf"], env["ps3"]
    w1, w2, b1, b2t, foldm = (env[k] for k in ("w1", "w2", "b1", "b2t", "foldm"))
    idxp1, idxal, zbuf, partial, outsb = (
        env[k] for k in ("idxp1", "idxal", "zbuf", "partial", "outsb"))
    mxall, seall, lsall, uall = (
        env[k] for k in ("mxall", "seall", "lsall", "uall"))
    F32L = F32

    def mm_chunks():
        for c0 in range(0, NPAD, MMCH):
            yield c0, min(MMCH, NPAD - c0)

    # ---- phase 0: h = x @ W1, zt = dinv*h -> zloc ------------------------
    for c0, cw in ([] if cfg.SKIP_PHASE0 else list(mm_chunks())):
        ph = psh.tile([H, MMCH], F32, tag="ph")
        for k in range(KT):
            xt = xp.tile([KP, MMCH], BF16, tag="xt")
            nc.sync.dma_start(xt[:, :cw], T["xT"][k * KP:(k + 1) * KP, c0:c0 + cw])
            nc.tensor.matmul(ph[:, :cw], w1[:, k * H:(k + 1) * H], xt[:, :cw],
                             start=(k == 0), stop=(k == KT - 1))
        dv = dp.tile([H, MMCH], F32, tag="dv")
        nc.sync.dma_start(dv[:, :cw], T["dinv"][:, c0:c0 + cw])
        zt = zc.tile([H, MMCH], F32, tag="zt")
        nc.vector.tensor_mul(zt[:, :cw], ph[:, :cw], dv[:, :cw])
        nc.sync.dma_start(T["zloc"][:, c0:c0 + cw], zt[:, :cw])
        ztb = zc.tile([H, MMCH], BF16, tag="ztb")
        nc.vector.tensor_copy(ztb[:, :cw], zt[:, :cw])
        nc.sync.dma_start(T["zlocb"][:, c0:c0 + cw], ztb[:, :cw])

    if cfg.FAKE_CC:
        for q in range(cfg.NCORES):
            nc.sync.dma_start(T["zallb"][16 * q:16 * (q + 1), :], T["zlocb"][:])
    else:
        nc.gpsimd.collective_compute(
            "AllGather", mybir.AluOpType.bypass,
            replica_groups=[list(range(cfg.NCORES))],
            ins=[T["zlocb"].opt()], outs=[T["zallb"].opt()],
        )

    for layer in range(2):
        zallb = T["zallb"] if layer == 0 else T["rallb"]
        # stage the bf16 gathered z through small chunks, widening to f32
        for z0 in range(0, NPAD, 2048):
            zw = min(2048, NPAD - z0)
            zb = alp.tile([128, 2048], BF16, tag="zb")
            nc.sync.dma_start(zb[:, :zw], zallb[:, z0:z0 + zw])
            nc.vector.tensor_copy(zbuf[:, z0:z0 + zw], zb[:, :zw])
        if cfg.DEBUG and layer == 0:
            nc.sync.dma_start(T["dbgz"][:], zbuf[:])
        if not cfg.SKIP_REDUCE:
            nc.vector.memset(partial[:, 0:1], 0.0)

        # ---- phase 1: per-edge gather + class-strided segment sums ------
        if cfg.DEBUG and layer == 0:
            nc.sync.dma_start(T["dbgi2"][:], idxp1[:])
        for mt_i, (i0, nidx, pcs) in enumerate(
                [] if cfg.SKIP_GATHER else st.megatiles):
            m = mg.tile([128, cfg.TILE], F32, tag="m")
            nc.gpsimd.ap_gather(
                m[:, :nidx], zbuf[:], idxp1[:, i0 // 16:(i0 + nidx) // 16],
                channels=128, num_elems=NPAD, d=1, num_idxs=nidx,
            )
            if cfg.DEBUG and layer == 0:
                nc.sync.dma_start(T["dbgm"][:, i0:i0 + nidx], m[:, :nidx])
            for (pp, k, mcnt, off) in ([] if cfg.SKIP_REDUCE else pcs):
                nc.vector.tensor_reduce(
                    partial[:, pp:pp + mcnt],
                    m[:, off:off + mcnt * k].rearrange("p (a b) -> p a b", b=k),
                    axis=mybir.AxisListType.X, op=mybir.AluOpType.add,
                )

        if cfg.DEBUG and layer == 0:
            nc.sync.dma_start(T["dbgi"][:], idxp1[:])
            nc.sync.dma_start(T["dbgz2"][:], zbuf[:])
            for c0 in ([] if cfg.SKIP_REDUCE else range(0, st.P_sz, cfg.TILE)):
                cw = min(cfg.TILE, st.P_sz - c0)
                dtile = mg.tile([128, cfg.TILE], F32, tag="m")
                nc.vector.tensor_copy(dtile[:, :cw], partial[:, c0:c0 + cw])
                nc.sync.dma_start(T["dbgp"][:, c0:c0 + cw], dtile[:, :cw])
        if cfg.SKIP_PHASE2:
            return

        # ---- phase 2: align octant partials, fold, pointwise ------------
        for a0 in range(0, NPAD, ALCH):
            aw = min(ALCH, NPAD - a0)
            at = alp.tile([128, ALCH], F32, tag="a")
            nc.gpsimd.ap_gather(
                at[:, :aw], partial[:], idxal[:, a0 // 16:(a0 + aw) // 16],
                channels=128, num_elems=st.P_sz, d=1, num_idxs=aw,
            )
            for c0 in range(a0, a0 + aw, MMCH):
                cw = min(MMCH, a0 + aw - c0)
                pf = psf.tile([H, MMCH], F32, tag="pf")
                nc.tensor.matmul(pf[:, :cw], foldm[:], at[:, c0 - a0:c0 - a0 + cw],
                                 start=True, stop=True)
                dv = dp.tile([H, MMCH], F32, tag="dv")
                nc.sync.dma_start(dv[:, :cw], T["dinv"][:, c0:c0 + cw])
                zsl = dp.tile([H, MMCH], F32, tag="zsl")
                nc.sync.dma_start(
                    zsl[:, :cw],
                    (T["zloc"] if layer == 0 else T["rloc"])[:, c0:c0 + cw])
                t0 = zc.tile([H, MMCH], F32, tag="t0")
                nc.vector.tensor_add(t0[:, :cw], pf[:, :cw], zsl[:, :cw])
                t1 = zc.tile([H, MMCH], F32, tag="t1")
                nc.vector.tensor_mul(t1[:, :cw], t0[:, :cw], dv[:, :cw])
                if layer == 0:
                    t2 = zc.tile([H, MMCH], F32, tag="t2")
                    nc.scalar.activation(t2[:, :cw], t1[:, :cw],
                                         mybir.ActivationFunctionType.Relu,
                                         bias=b1[:, 0:1])
                    t3 = zc.tile([H, MMCH], F32, tag="t3")
                    nc.vector.tensor_mul(t3[:, :cw], t2[:, :cw], dv[:, :cw])
                    nc.sync.dma_start(T["rloc"][:, c0:c0 + cw], t3[:, :cw])
                    t3b = zc.tile([H, MMCH], BF16, tag="t3b")
                    nc.vector.tensor_copy(t3b[:, :cw], t3[:, :cw])
                    nc.sync.dma_start(T["rlocb"][:, c0:c0 + cw], t3b[:, :cw])
                else:
                    # t1 = dinv*agg2 ; logits = t1.T @ W2 + b2 into outsb;
                    # per-block max into mxall (log_softmax finished after
                    # the chunk loop so each activation table loads once)
                    for s0 in range(0, cw, 128):
                        sw = min(128, cw - s0)
                        blk = (c0 + s0) // 128
                        p3 = ps3.tile([128, CLS], F32, tag="p3")
                        nc.tensor.matmul(p3[:sw, :], t1[:, s0:s0 + sw], w2[:],
                                         start=True, stop=True)
                        nc.vector.tensor_add(
                            outsb[:sw, blk * CLS:(blk + 1) * CLS],
                            p3[:sw, :], b2t[:sw, :])
                        nc.vector.tensor_reduce(
                            mxall[:sw, blk:blk + 1],
                            outsb[:sw, blk * CLS:(blk + 1) * CLS],
                            axis=mybir.AxisListType.X,
                            op=mybir.AluOpType.max, negate=True)

        if layer == 1:
            # batched log_softmax epilogue: all Exp, then one Ln, then adds
            for b in range(NBLK):
                ex = sm.tile([128, CLS], F32, tag="ex")
                nc.scalar.activation(ex[:], outsb[:, b * CLS:(b + 1) * CLS],
                                     mybir.ActivationFunctionType.Exp,
                                     bias=mxall[:, b:b + 1],
                                     accum_out=seall[:, b:b + 1])
            nc.scalar.activation(lsall[:], seall[:],
                                 mybir.ActivationFunctionType.Ln)
            nc.vector.tensor_sub(uall[:], mxall[:], lsall[:])
            for b in range(NBLK):
                nc.vector.tensor_scalar_add(
                    outsb[:, b * CLS:(b + 1) * CLS],
                    outsb[:, b * CLS:(b + 1) * CLS], uall[:, b:b + 1])

        if layer == 0:
            if cfg.DEBUG:
                nc.sync.dma_start(T["dbgr"][:], T["rloc"][:])
            if cfg.FAKE_CC:
                for q in range(cfg.NCORES):
                    nc.sync.dma_start(T["rallb"][16 * q:16 * (q + 1), :],
                                      T["rlocb"][:])
            else:
                nc.gpsimd.collective_compute(
                    "AllGather", mybir.AluOpType.bypass,
                    replica_groups=[list(range(cfg.NCORES))],
                    ins=[T["rlocb"].opt()], outs=[T["rallb"].opt()],
                )

    nc.sync.dma_start(
        T["out"].rearrange("(a p) k -> p a k", p=128),
        outsb[:].rearrange("p (a k) -> p a k", k=CLS))


def build_program(cfg: Cfg, st: Struct):
    nc = bacc.Bacc("TRN2", target_bir_lowering=False, debug=False,
                   enable_asserts=False, num_devices=cfg.NCORES)
    H, CLS, NPAD = cfg.H, cfg.CLS, cfg.NPAD
    T = {}
    T["xT"] = nc.dram_tensor("xT", [cfg.KT * cfg.KP, NPAD], BF16,
                             kind="ExternalInput").ap()
    T["W1s"] = nc.dram_tensor("W1s", [cfg.KP, cfg.KT * H], BF16,
                              kind="ExternalInput").ap()
    T["W2s"] = nc.dram_tensor("W2s", [H, CLS], F32, kind="ExternalInput").ap()
    T["b1s"] = nc.dram_tensor("b1s", [H, 1], F32, kind="ExternalInput").ap()
    T["b2t"] = nc.dram_tensor("b2t", [128, CLS], F32, kind="ExternalInput").ap()
    T["dinv"] = nc.dram_tensor("dinv", [H, NPAD], F32,
                               kind="ExternalInput").ap()
    T["foldm"] = nc.dram_tensor("foldm", [128, H], F32,
                                kind="ExternalInput").ap()
    T["idxp1"] = nc.dram_tensor("idxp1", [128, st.S // 16], I16,
                                kind="ExternalInput").ap()
    T["idxal"] = nc.dram_tensor("idxal", [128, NPAD // 16], I16,
                                kind="ExternalInput").ap()
    T["zloc"] = nc.dram_tensor(
        "zloc", [H, NPAD], F32,
        **({"kind": "ExternalInput"} if cfg.SKIP_PHASE0 else {})).ap()
    T["zlocb"] = nc.dram_tensor("zlocb", [H, NPAD], BF16).ap()
    T["zallb"] = nc.dram_tensor("zallb", [128, NPAD], BF16,
                                addr_space="Shared").ap()
    T["rloc"] = nc.dram_tensor("rloc", [H, NPAD], F32).ap()
    T["rlocb"] = nc.dram_tensor("rlocb", [H, NPAD], BF16).ap()
    T["rallb"] = nc.dram_tensor("rallb", [128, NPAD], BF16,
                                addr_space="Shared").ap()
    T["out"] = nc.dram_tensor("out", [NPAD, CLS], F32, kind="ExternalOutput").ap()
    if cfg.DEBUG:
        T["dbgp"] = nc.dram_tensor("dbgp", [128, st.P_sz], F32,
                                   kind="ExternalOutput").ap()
        T["dbgr"] = nc.dram_tensor("dbgr", [H, NPAD], F32,
                                   kind="ExternalOutput").ap()
        T["dbgm"] = nc.dram_tensor("dbgm", [128, st.S], F32,
                                   kind="ExternalOutput").ap()
        T["dbgi"] = nc.dram_tensor("dbgi", [128, st.S // 16], I16,
                                   kind="ExternalOutput").ap()
        T["dbgi2"] = nc.dram_tensor("dbgi2", [128, st.S // 16], I16,
                                    kind="ExternalOutput").ap()
        T["dbgz"] = nc.dram_tensor("dbgz", [128, NPAD], F32,
                                   kind="ExternalOutput").ap()
        T["dbgz2"] = nc.dram_tensor("dbgz2", [128, NPAD], F32,
                                    kind="ExternalOutput").ap()

    with tile.TileContext(nc) as tc:
        gcn_body(tc, T, cfg, st)
    nc.compile()
    return nc


def make_inmaps(x, W1, b1, W2, b2, cfg: Cfg, st: Struct):
    N, NSH, NPAD, H = cfg.N, cfg.NSH, cfg.NPAD, cfg.H
    FPAD = cfg.KT * cfg.KP
    W1p = np.zeros((FPAD, H), np.float32)
    W1p[:cfg.FIN] = np.asarray(W1, np.float32)
    W1s = W1p.reshape(cfg.KT, cfg.KP, H).transpose(1, 0, 2).reshape(
        cfg.KP, cfg.KT * H)
    b2t = np.tile(np.asarray(b2, np.float32)[None, :], (128, 1))
    foldm = np.zeros((128, H), np.float32)
    for q in range(cfg.NCORES):
        foldm[16 * q + np.arange(H) % 16, np.arange(H)] = 1.0
    import ml_dtypes
    W1s = W1s.astype(ml_dtypes.bfloat16)
    in_maps = []
    for n in range(cfg.NCORES):
        xs = np.asarray(x[n * NSH:(n + 1) * NSH], np.float32)
        xT = np.zeros((FPAD, NPAD), ml_dtypes.bfloat16)
        xT[:cfg.FIN, :NSH] = xs.T.astype(ml_dtypes.bfloat16)
        dv = np.zeros((H, NPAD), np.float32)
        dv[:, :NSH] = st.dinv[n * NSH:(n + 1) * NSH][None, :]
        in_maps.append({
            "xT": xT, "W1s": W1s,
            "W2s": np.asarray(W2, np.float32),
            "b1s": np.asarray(b1, np.float32).reshape(H, 1),
            "b2t": b2t, "dinv": dv, "foldm": foldm,
            "idxp1": st.idxp1[n], "idxal": st.idxal[n],
        })
    return in_maps


# --------------------------------------------------------------------------
# public entry point
# --------------------------------------------------------------------------
PROFILE = False          # set True to capture an NTFF trace / exec time
LAST_RESULT = None       # BassKernelResults of the last run


def kernel(x, edge_index, W1, b1, W2, b2):
    global LAST_RESULT
    cfg = Cfg()
    x = np.asarray(x)
    assert x.shape == (cfg.N, cfg.FIN), x.shape
    st = preprocess(np.asarray(edge_index), cfg)
    nc = build_program(cfg, st)
    in_maps = make_inmaps(x, W1, b1, W2, b2, cfg, st)
    from concourse.bass_utils import run_bass_kernel_spmd
    res = run_bass_kernel_spmd(nc, in_maps, list(range(cfg.NCORES)),
                               trace=PROFILE)
    LAST_RESULT = res
    out = np.concatenate(
        [res.results[n]["out"][:cfg.NSH] for n in range(cfg.NCORES)], axis=0)
    return out.astype(np.float32)

